# revision 1
# baseline (speedup 1.0000x reference)
"""2-layer GCN (GCNConv -> BatchNorm(train) -> ReLU -> GCNConv -> ReLU) on 8 TRN2
NeuronCores, SPMD (one NEFF on all cores).

Sharding: nodes padded 100000 -> 102400 = 8*12800, core i owns rows
[i*12800,(i+1)*12800); edges partitioned by dst owner so the segment-sum scatter
is local; small 128x128 weights replicated; the layer-2 feature table is
assembled with an AllGather; BatchNorm stats with a 1KB AllReduce.

Per-core pipeline:
  A) H1s table = (dis*x)@W1 for ALL nodes (replicated compute, node-major f32
     in local DRAM); self-loop term dis^2*(x@W1) for own rows seeds the
     aggregation accumulator.
  B) per-edge rows fetched with gpsimd.dma_gather (int16 idx over 4 base-offset
     blocks of 32768 rows, 4 SWDGE queues); segment-sum scatter = one-hot
     matmul accumulated in PSUM per 128-dst chunk.  dis[src]*dis[dst] is
     separable: tables carry the src factor, output rows the dst factor.
     b1 is dropped (BatchNorm output is invariant to a pre-BN bias).
  C) BN stats via ones-matmul partition reduction + AllReduce; affine+ReLU
     fused into one scalar-engine activation in transposed space.
  D) H2s own rows = dis*(h2@W2); AllGather -> full layer-2 table.
  E) same gather/scatter for layer 2 (+b2, ReLU) -> own output rows.
"""
import numpy as np

import concourse.bass as bass
import concourse.mybir as mybir
import concourse.tile as tile
from concourse import bacc
from concourse.bass_utils import run_bass_kernel_spmd
from concourse.masks import make_identity

N = 100000
F = 128
NCORES = 8
NPAD = 102400
OWN = NPAD // NCORES          # 12800
CHUNKS = OWN // 128           # 100
BLK = 32768
NBLK = 4
BN_EPS = 1e-5
GROUPS = NPAD // 128          # 800
MAX_IDX_PER_CALL = 1024

LAST_EXEC_NS = None
_cache = {}


def _prep(x, edge_index):
    src = np.asarray(edge_index[0]).astype(np.int64)
    dst = np.asarray(edge_index[1]).astype(np.int64)

    deg = np.bincount(dst, minlength=N).astype(np.float32) + 1.0
    dis = np.zeros(NPAD, dtype=np.float32)
    dis[:N] = 1.0 / np.sqrt(deg)

    xs = np.zeros((NPAD, F), dtype=np.float32)
    xs[:N] = np.asarray(x, dtype=np.float32) * dis[:N, None]
    xsT = np.ascontiguousarray(xs.T)                       # [128, NPAD]

    owner = dst // OWN
    chunk = (dst % OWN) // 128
    blk = src // BLK
    cell = ((owner * CHUNKS + chunk) * NBLK + blk).astype(np.int64)
    order = np.argsort(cell, kind="stable")
    cell_s = cell[order]
    src_s = src[order]
    dstloc_s = (dst[order] % 128).astype(np.float32)

    counts = np.bincount(cell_s, minlength=NCORES * CHUNKS * NBLK)
    counts = counts.reshape(NCORES, CHUNKS, NBLK)
    C = counts.max(axis=0)
    C = ((C + 127) // 128) * 128
    C = np.maximum(C, 128)
    slots_per_chunk = C.sum(axis=1)
    tot_slots = int(slots_per_chunk.sum())
    ntiles = tot_slots // 128

    cell_off = np.zeros((CHUNKS, NBLK), dtype=np.int64)
    cell_off.reshape(-1)[1:] = np.cumsum(C.reshape(-1))[:-1]

    starts = np.zeros(NCORES * CHUNKS * NBLK + 1, dtype=np.int64)
    starts[1:] = np.cumsum(counts.reshape(-1))

    per_core = []
    for i in range(NCORES):
        srcidx = np.zeros(tot_slots, dtype=np.int16)          # pads gather row 0
        dstloc = np.full(tot_slots, -1.0, dtype=np.float32)   # pads hit no column
        for c in range(CHUNKS):
            for b in range(NBLK):
                k = (i * CHUNKS + c) * NBLK + b
                n = int(counts[i, c, b])
                o = int(cell_off[c, b])
                if n:
                    sl = slice(starts[k], starts[k] + n)
                    srcidx[o:o + n] = (src_s[sl] - b * BLK).astype(np.int16)
                    dstloc[o:o + n] = dstloc_s[sl]
        iw = srcidx.reshape(tot_slots // 16, 16).T            # [16, tot/16]
        srcidx_w = np.ascontiguousarray(np.tile(iw, (8, 1)))  # [128, tot/16]
        dstloc_t = np.ascontiguousarray(dstloc.reshape(ntiles, 128).T)
        disT = np.ascontiguousarray(
            dis[i * OWN:(i + 1) * OWN].reshape(CHUNKS, 128).T)
        xs_ownT = np.ascontiguousarray(xsT[:, i * OWN:(i + 1) * OWN])
        per_core.append({"srcidx": srcidx_w, "dstloc": dstloc_t,
                         "disT": disT, "xs_ownT": xs_ownT})

    consts = {"C": C, "cell_off": cell_off, "tot_slots": tot_slots,
              "ntiles": ntiles, "slots_per_chunk": slots_per_chunk}
    return consts, xsT, per_core


def _build(consts):
    C = consts["C"]
    cell_off = consts["cell_off"]
    tot_slots = consts["tot_slots"]
    ntiles = consts["ntiles"]
    spc = consts["slots_per_chunk"]

    f32 = mybir.dt.float32
    AF = mybir.ActivationFunctionType
    OP = mybir.AluOpType
    nc = bacc.Bacc("TRN2", target_bir_lowering=False, debug=False,
                   num_devices=NCORES, num_swdge_queues=4)

    xsT_d = nc.dram_tensor("xsT", [F, NPAD], f32, kind="ExternalInput").ap()
    xso_d = nc.dram_tensor("xs_ownT", [F, OWN], f32, kind="ExternalInput").ap()
    W1_d = nc.dram_tensor("W1", [F, F], f32, kind="ExternalInput").ap()
    W2_d = nc.dram_tensor("W2", [F, F], f32, kind="ExternalInput").ap()
    gamma_d = nc.dram_tensor("gamma_c", [F, 1], f32, kind="ExternalInput").ap()
    beta_d = nc.dram_tensor("beta_c", [F, 1], f32, kind="ExternalInput").ap()
    b2m_d = nc.dram_tensor("b2_mat", [128, F], f32, kind="ExternalInput").ap()
    disT_d = nc.dram_tensor("disT", [128, CHUNKS], f32, kind="ExternalInput").ap()
    srcidx_d = nc.dram_tensor("srcidx", [128, tot_slots // 16], mybir.dt.int16,
                              kind="ExternalInput").ap()
    dstloc_d = nc.dram_tensor("dstloc", [128, ntiles], f32,
                              kind="ExternalInput").ap()
    out_d = nc.dram_tensor("out", [OWN, F], f32, kind="ExternalOutput").ap()

    h1s_t = nc.dram_tensor("h1s_tab", [NPAD, F], f32)
    ag_in = nc.dram_tensor("ag_in", [OWN, F], f32)
    ag_out = nc.dram_tensor("ag_out", [NPAD, F], f32, addr_space="Shared")
    bn_in = nc.dram_tensor("bn_in", [F, 2], f32)
    bn_out = nc.dram_tensor("bn_out", [F, 2], f32, addr_space="Shared")

    with tile.TileContext(nc) as tc:
        with tc.tile_pool(name="const", bufs=1) as constp, \
             tc.tile_pool(name="big", bufs=1) as bigp, \
             tc.tile_pool(name="xs", bufs=4) as xsp, \
             tc.tile_pool(name="h", bufs=4) as hp, \
             tc.tile_pool(name="psg", bufs=2, space="PSUM") as psg, \
             tc.tile_pool(name="psb", bufs=4, space="PSUM") as psb, \
             tc.tile_pool(name="pss", bufs=1, space="PSUM") as pss, \
             tc.tile_pool(name="gbuf", bufs=3) as gbufp, \
             tc.tile_pool(name="oh", bufs=8) as ohp, \
             tc.tile_pool(name="wk", bufs=4) as wp, \
             tc.tile_pool(name="st", bufs=1) as stp:

            # ---- constants ----
            W1_t = constp.tile([F, F], f32)
            W2_t = constp.tile([F, F], f32)
            ident = constp.tile([128, 128], f32)
            iota_r = constp.tile([128, 128], f32)
            ones_c = constp.tile([128, 1], f32)
            gamma_t = constp.tile([F, 1], f32)
            beta_t = constp.tile([F, 1], f32)
            b2m_t = constp.tile([128, F], f32)
            disT_t = constp.tile([128, CHUNKS], f32)
            nc.sync.dma_start(out=W1_t[:], in_=W1_d[:])
            nc.sync.dma_start(out=W2_t[:], in_=W2_d[:])
            nc.sync.dma_start(out=gamma_t[:], in_=gamma_d[:])
            nc.sync.dma_start(out=beta_t[:], in_=beta_d[:])
            nc.sync.dma_start(out=b2m_t[:], in_=b2m_d[:])
            nc.sync.dma_start(out=disT_t[:], in_=disT_d[:])
            make_identity(nc, ident[:])
            iota_i = constp.tile([128, 128], mybir.dt.int32)
            nc.gpsimd.iota(iota_i[:], pattern=[[1, 128]], base=0,
                           channel_multiplier=0)
            nc.vector.tensor_copy(out=iota_r[:], in_=iota_i[:])
            nc.vector.memset(ones_c[:], 1.0)

            srcidx_sb = bigp.tile([128, tot_slots // 16], mybir.dt.int16)
            dstloc_sb = bigp.tile([128, ntiles], f32)
            nc.sync.dma_start(out=srcidx_sb[:], in_=srcidx_d[:])
            nc.sync.dma_start(out=dstloc_sb[:], in_=dstloc_d[:])

            agg = bigp.tile([128, CHUNKS, 128], f32)

            # ---- Phase A: full H1s table (batches of 16 node groups) ----
            BG = 16
            for gg in range(GROUPS // BG):
                xs_t = xsp.tile([F, BG * 128], f32, tag="xs")
                nc.sync.dma_start(
                    out=xs_t[:],
                    in_=xsT_d[:, gg * BG * 128:(gg + 1) * BG * 128])
                hblk = hp.tile([128, BG, F], f32, tag="h")
                for k in range(BG):
                    ps = psg.tile([128, F], f32, tag="g")
                    nc.tensor.matmul(out=ps[:],
                                     lhsT=xs_t[:, k * 128:(k + 1) * 128],
                                     rhs=W1_t[:], start=True, stop=True)
                    nc.vector.tensor_copy(out=hblk[:, k, :], in_=ps[:])
                nc.sync.dma_start(
                    out=h1s_t[gg * BG * 128:(gg + 1) * BG * 128, :]
                        .rearrange("(k p) f -> p k f", p=128),
                    in_=hblk[:])

            # ---- Phase A2: layer-1 self term (own rows) ----
            for c in range(CHUNKS):
                xs_t = xsp.tile([F, 128], f32, tag="xs")
                nc.sync.dma_start(out=xs_t[:], in_=xso_d[:, c * 128:(c + 1) * 128])
                ps = psg.tile([128, F], f32, tag="g")
                nc.tensor.matmul(out=ps[:], lhsT=xs_t[:], rhs=W1_t[:],
                                 start=True, stop=True)
                nc.vector.tensor_scalar_mul(out=agg[:, c, :], in0=ps[:],
                                            scalar1=disT_t[:, c:c + 1])

            # ---- shared gather/scatter pass ----
            def layer_pass(table, out_stage):
                qn = 0
                for c in range(CHUNKS):
                    nb = int(spc[c]) // 128
                    gb = gbufp.tile([128, nb, 128], f32, tag="gb")
                    base_o = int(cell_off[c, 0])
                    for b in range(NBLK):
                        cnt = int(C[c, b])
                        o = int(cell_off[c, b])
                        lo = b * BLK
                        hi = min(NPAD, lo + BLK)
                        for sub in range(0, cnt, MAX_IDX_PER_CALL):
                            n = min(MAX_IDX_PER_CALL, cnt - sub)
                            ol = o - base_o + sub
                            nc.gpsimd.dma_gather(
                                gb[:, ol // 128:(ol + n) // 128, :],
                                table[lo:hi, :],
                                srcidx_sb[:, (o + sub) // 16:(o + sub + n) // 16],
                                n, n, F, queue_num=qn)
                            qn = (qn + 1) % 4
                    ps = psb.tile([128, F], f32, tag="acc")
                    base_t = base_o // 128
                    for t in range(nb):
                        oh = ohp.tile([128, 128], f32, tag="oh")
                        nc.vector.tensor_tensor(
                            out=oh[:],
                            in0=dstloc_sb[:, base_t + t:base_t + t + 1]
                                .to_broadcast([128, 128]),
                            in1=iota_r[:],
                            op=OP.is_equal)
                        nc.tensor.matmul(out=ps[:], lhsT=oh[:],
                                         rhs=gb[:, t, :],
                                         start=(t == 0), stop=(t == nb - 1))
                    out_stage(c, ps)

            # ---- Phase B: layer-1 scatter (accumulate onto self term) ----
            def b_stage(c, ps):
                t = wp.tile([128, 128], f32, tag="bs")
                nc.vector.tensor_scalar_mul(out=t[:], in0=ps[:],
                                            scalar1=disT_t[:, c:c + 1])
                nc.vector.tensor_tensor(out=agg[:, c, :], in0=t[:],
                                        in1=agg[:, c, :], op=OP.add)
            layer_pass(h1s_t.ap(), b_stage)

            # ---- Phase C: BN stats + AllReduce ----
            sum_ps = pss.tile([128, 1], f32, tag="s0")
            for c in range(CHUNKS):
                nc.tensor.matmul(out=sum_ps[:], lhsT=agg[:, c, :], rhs=ones_c[:],
                                 start=(c == 0), stop=(c == CHUNKS - 1))
            sq_ps = pss.tile([128, 1], f32, tag="s1")
            for c in range(CHUNKS):
                sq_t = wp.tile([128, 128], f32, tag="sq")
                nc.vector.tensor_tensor(out=sq_t[:], in0=agg[:, c, :],
                                        in1=agg[:, c, :], op=OP.mult)
                nc.tensor.matmul(out=sq_ps[:], lhsT=sq_t[:], rhs=ones_c[:],
                                 start=(c == 0), stop=(c == CHUNKS - 1))
            stats = stp.tile([128, 2], f32)
            nc.vector.tensor_copy(out=stats[:, 0:1], in_=sum_ps[:])
            nc.vector.tensor_copy(out=stats[:, 1:2], in_=sq_ps[:])
            nc.sync.dma_start(out=bn_in[:], in_=stats[:])
            nc.gpsimd.collective_compute(
                "AllReduce", OP.add, ins=[bn_in.ap()], outs=[bn_out.ap()],
                replica_groups=[list(range(NCORES))])
            gstats = stp.tile([128, 2], f32)
            nc.sync.dma_start(out=gstats[:], in_=bn_out[:])

            mean_t = stp.tile([128, 1], f32)
            ex2_t = stp.tile([128, 1], f32)
            var_t = stp.tile([128, 1], f32)
            sd_t = stp.tile([128, 1], f32)
            rstd_t = stp.tile([128, 1], f32)
            scale_c = stp.tile([128, 1], f32)
            shift_c = stp.tile([128, 1], f32)
            nc.vector.tensor_scalar_mul(out=mean_t[:], in0=gstats[:, 0:1],
                                        scalar1=1.0 / N)
            nc.vector.tensor_scalar_mul(out=ex2_t[:], in0=gstats[:, 1:2],
                                        scalar1=1.0 / N)
            nc.vector.tensor_tensor(out=var_t[:], in0=mean_t[:], in1=mean_t[:],
                                    op=OP.mult)
            nc.vector.tensor_tensor(out=var_t[:], in0=ex2_t[:], in1=var_t[:],
                                    op=OP.subtract)
            eps_t = stp.tile([128, 1], f32)
            nc.vector.memset(eps_t[:], BN_EPS)
            nc.scalar.activation(sd_t[:], var_t[:], AF.Sqrt, bias=eps_t[:])
            nc.vector.reciprocal(out=rstd_t[:], in_=sd_t[:])
            nc.vector.tensor_tensor(out=scale_c[:], in0=rstd_t[:], in1=gamma_t[:],
                                    op=OP.mult)
            nc.vector.tensor_tensor(out=shift_c[:], in0=mean_t[:], in1=scale_c[:],
                                    op=OP.mult)
            nc.vector.tensor_tensor(out=shift_c[:], in0=beta_t[:], in1=shift_c[:],
                                    op=OP.subtract)

            # ---- Phase D: h2 own rows, H2s table rows, layer-2 self term ----
            for c in range(CHUNKS):
                trps = psg.tile([128, 128], f32, tag="g")
                nc.tensor.transpose(out=trps[:], in_=agg[:, c, :],
                                    identity=ident[:])
                h2inT = wp.tile([128, 128], f32, tag="h2")
                nc.scalar.activation(h2inT[:], trps[:], AF.Relu,
                                     bias=shift_c[:], scale=scale_c[:])
                ps2 = psg.tile([128, 128], f32, tag="g")
                nc.tensor.matmul(out=ps2[:], lhsT=h2inT[:], rhs=W2_t[:],
                                 start=True, stop=True)
                h2s_t = hp.tile([128, F], f32, tag="h")
                nc.vector.tensor_scalar_mul(out=h2s_t[:], in0=ps2[:],
                                            scalar1=disT_t[:, c:c + 1])
                nc.sync.dma_start(out=ag_in[c * 128:(c + 1) * 128, :],
                                  in_=h2s_t[:])
                nc.vector.tensor_scalar_mul(out=agg[:, c, :], in0=h2s_t[:],
                                            scalar1=disT_t[:, c:c + 1])

            nc.gpsimd.collective_compute(
                "AllGather", OP.bypass, ins=[ag_in.ap()], outs=[ag_out.ap()],
                replica_groups=[list(range(NCORES))])

            # ---- Phase E: layer-2 scatter + bias + relu + output ----
            def e_stage(c, ps):
                t = wp.tile([128, 128], f32, tag="eo")
                nc.vector.tensor_scalar_mul(out=t[:], in0=ps[:],
                                            scalar1=disT_t[:, c:c + 1])
                nc.vector.tensor_tensor(out=t[:], in0=t[:], in1=agg[:, c, :],
                                        op=OP.add)
                nc.vector.tensor_tensor(out=t[:], in0=t[:], in1=b2m_t[:],
                                        op=OP.add)
                nc.scalar.activation(t[:], t[:], AF.Relu)
                nc.sync.dma_start(out=out_d[c * 128:(c + 1) * 128, :], in_=t[:])
            layer_pass(ag_out.ap(), e_stage)

    nc.compile()
    return nc


def kernel(**inputs):
    global LAST_EXEC_NS
    import os
    x = inputs["x"]
    W1 = np.asarray(inputs["W1"], dtype=np.float32)
    W2 = np.asarray(inputs["W2"], dtype=np.float32)
    gamma = np.asarray(inputs["gamma"], dtype=np.float32)
    beta = np.asarray(inputs["beta"], dtype=np.float32)
    b2 = np.asarray(inputs["b2"], dtype=np.float32)
    edge_index = inputs["edge_index"]

    key = (hash(np.asarray(edge_index)[:, ::997].tobytes()),)
    if key not in _cache:
        consts, xsT, per_core = _prep(x, edge_index)
        nc = _build(consts)
        _cache[key] = (consts, nc)
    else:
        consts, nc = _cache[key]
        _, xsT, per_core = _prep(x, edge_index)

    shared = {
        "xsT": xsT,
        "W1": W1, "W2": W2,
        "gamma_c": gamma.reshape(F, 1).copy(),
        "beta_c": beta.reshape(F, 1).copy(),
        "b2_mat": np.ascontiguousarray(np.broadcast_to(b2.reshape(1, F),
                                                       (128, F))),
    }
    in_maps = []
    for i in range(NCORES):
        m = dict(shared)
        m.update(per_core[i])
        in_maps.append(m)

    trace = bool(os.environ.get("BASS_GCN_TRACE"))
    res = run_bass_kernel_spmd(nc, in_maps, list(range(NCORES)), trace=trace)
    LAST_EXEC_NS = res.exec_time_ns

    out = np.concatenate([res.results[i]["out"] for i in range(NCORES)], axis=0)
    return np.ascontiguousarray(out[:N]).astype(np.float32)



# revision 13
# speedup vs baseline: 1.8706x; 1.8706x over previous
"""2-layer GCN (GCNConv -> BatchNorm(train) -> ReLU -> GCNConv -> ReLU) on 8 TRN2
NeuronCores, SPMD (one NEFF on all cores).

v2 design (aggregate-then-matmul, bf16 datapath):
  - Host provides xsn = (dis*x) as a bf16 [NPAD,128] gather table; layer-1
    gathers xsn rows per edge directly (no H1-table build phase at all) and
    the W1 matmul is applied per dst-chunk AFTER aggregation.
  - Edges partitioned by dst owner, sorted by (chunk, src-block, src); the
    segment-sum scatter is a one-hot matmul per 128-row tile producing the
    aggregate TRANSPOSED ([fin, dst]) so the @W chain needs no transposes.
  - Padding slots carry idx=-1: the DMAGatherAnt ucode skips trailing
    negative indices per call, so padding costs no Q7 descriptor-gen time
    and no DMA bytes (except the first 3 chunks, which zero-pad to
    initialize the rotating gather buffers).
  - Self-loop terms are folded in as an extra accumulating matmul with a
    feature-major own-shard slice as lhsT.
  - BN stats via ones-matmuls accumulated across chunks + 1KB AllReduce;
    scale/shift folded as z = relu(dis * (scale_mat*h1 + shift_mat)) with
    scale/shift broadcast matrices built by K=1 outer-product matmuls.
  - z table (bf16) AllGathered, bulk-copied Shared->local DRAM, then
    layer 2 runs the same gather/scatter machinery; W2 + self term + b2 +
    relu applied per chunk; f32 output rows.
"""
import numpy as np
import ml_dtypes

import concourse.bass as bass
import concourse.mybir as mybir
import concourse.tile as tile
from concourse import bacc
from concourse.bass_utils import run_bass_kernel_spmd
from concourse.masks import make_identity

N = 100000
F = 128
NCORES = 8
NPAD = 102400
OWN = NPAD // NCORES          # 12800
CHUNKS = OWN // 128           # 100
BLK = 32768
NBLK = 4
BN_EPS = 1e-5
MAX_IDX_PER_CALL = 2048
ZPAD_CHUNKS = 3               # first chunks zero-pad (init gather bufs)

BF16 = ml_dtypes.bfloat16

LAST_EXEC_NS = None
_cache = {}


def _prep(x, edge_index):
    src = np.asarray(edge_index[0]).astype(np.int64)
    dst = np.asarray(edge_index[1]).astype(np.int64)

    deg = np.bincount(dst, minlength=N).astype(np.float32) + 1.0
    dis = np.zeros(NPAD, dtype=np.float32)
    dis[:N] = 1.0 / np.sqrt(deg)

    xsn = np.zeros((NPAD, F), dtype=np.float32)
    xsn[:N] = np.asarray(x, dtype=np.float32) * dis[:N, None]
    xsn_bf = xsn.astype(BF16)

    owner = dst // OWN
    chunk = (dst % OWN) // 128
    blk = src // BLK
    cell = ((owner * CHUNKS + chunk) * NBLK + blk).astype(np.int64)
    order = np.lexsort((src, cell))
    cell_s = cell[order]
    src_s = src[order]
    dstloc_s = (dst[order] % 128).astype(np.float32)

    counts = np.bincount(cell_s, minlength=NCORES * CHUNKS * NBLK)
    counts = counts.reshape(NCORES, CHUNKS, NBLK)
    C = counts.max(axis=0)
    C = ((C + 127) // 128) * 128
    C = np.maximum(C, 128)
    slots_per_chunk = C.sum(axis=1)
    tot_slots = int(slots_per_chunk.sum())
    ntiles = tot_slots // 128

    cell_off = np.zeros((CHUNKS, NBLK), dtype=np.int64)
    cell_off.reshape(-1)[1:] = np.cumsum(C.reshape(-1))[:-1]

    starts = np.zeros(NCORES * CHUNKS * NBLK + 1, dtype=np.int64)
    starts[1:] = np.cumsum(counts.reshape(-1))

    per_core = []
    for i in range(NCORES):
        srcidx = np.full(tot_slots, -1, dtype=np.int16)
        dstloc = np.full(tot_slots, -1.0, dtype=np.float32)
        for c in range(CHUNKS):
            for b in range(NBLK):
                k = (i * CHUNKS + c) * NBLK + b
                n = int(counts[i, c, b])
                o = int(cell_off[c, b])
                if c < ZPAD_CHUNKS:
                    srcidx[o:o + C[c, b]] = 0   # zero-pad: real row-0 gathers
                if n:
                    sl = slice(starts[k], starts[k] + n)
                    srcidx[o:o + n] = (src_s[sl] - b * BLK).astype(np.int16)
                    dstloc[o:o + n] = dstloc_s[sl]
        iw = srcidx.reshape(tot_slots // 16, 16).T            # [16, tot/16]
        srcidx_w = np.ascontiguousarray(np.tile(iw, (8, 1)))  # [128, tot/16]
        dstloc_t = np.ascontiguousarray(
            dstloc.reshape(ntiles, 128).T).astype(BF16)
        disT = np.ascontiguousarray(
            dis[i * OWN:(i + 1) * OWN].reshape(CHUNKS, 128).T)
        xsoT = np.ascontiguousarray(
            xsn_bf[i * OWN:(i + 1) * OWN].T)                  # [F, OWN] bf16
        per_core.append({"srcidx": srcidx_w, "dstloc": dstloc_t,
                         "disT": disT, "xsoT": xsoT})

    ncalls = 0
    for c in range(CHUNKS):
        for b in range(NBLK):
            for sub in range(0, int(C[c, b]), MAX_IDX_PER_CALL):
                ncalls += 1
    for i in range(NCORES):
        ncnt = np.zeros((1, ncalls), dtype=np.int32)
        k = 0
        for c in range(CHUNKS):
            for b in range(NBLK):
                cnt = int(C[c, b])
                nreal = cnt if c < ZPAD_CHUNKS else int(counts[i, c, b])
                for sub in range(0, cnt, MAX_IDX_PER_CALL):
                    n = min(MAX_IDX_PER_CALL, cnt - sub)
                    ncnt[0, k] = min(max(nreal - sub, 0), n)
                    k += 1
        per_core[i]["ncnt"] = ncnt

    consts = {"C": C, "cell_off": cell_off, "tot_slots": tot_slots,
              "ntiles": ntiles, "slots_per_chunk": slots_per_chunk,
              "ncalls": ncalls}
    return consts, xsn_bf, per_core


def _build(consts):
    C = consts["C"]
    cell_off = consts["cell_off"]
    tot_slots = consts["tot_slots"]
    ntiles = consts["ntiles"]
    spc = consts["slots_per_chunk"]

    f32 = mybir.dt.float32
    bf16 = mybir.dt.bfloat16
    AF = mybir.ActivationFunctionType
    OP = mybir.AluOpType
    nc = bacc.Bacc("TRN2", target_bir_lowering=False, debug=False,
                   num_devices=NCORES, num_swdge_queues=4)

    xsn_d = nc.dram_tensor("xsn", [NPAD, F], bf16, kind="ExternalInput").ap()
    xsoT_d = nc.dram_tensor("xsoT", [F, OWN], bf16, kind="ExternalInput").ap()
    W1_d = nc.dram_tensor("W1bf", [F, F], bf16, kind="ExternalInput").ap()
    W2_d = nc.dram_tensor("W2bf", [F, F], bf16, kind="ExternalInput").ap()
    gamma_d = nc.dram_tensor("gamma_c", [F, 1], f32, kind="ExternalInput").ap()
    beta_d = nc.dram_tensor("beta_c", [F, 1], f32, kind="ExternalInput").ap()
    b2r_d = nc.dram_tensor("b2_row", [1, F], f32, kind="ExternalInput").ap()
    disT_d = nc.dram_tensor("disT", [128, CHUNKS], f32, kind="ExternalInput").ap()
    srcidx_d = nc.dram_tensor("srcidx", [128, tot_slots // 16], mybir.dt.int16,
                              kind="ExternalInput").ap()
    dstloc_d = nc.dram_tensor("dstloc", [128, ntiles], bf16,
                              kind="ExternalInput").ap()
    ncnt_d = nc.dram_tensor("ncnt", [1, consts["ncalls"]], mybir.dt.int32,
                            kind="ExternalInput").ap()
    out_d = nc.dram_tensor("out", [OWN, F], f32, kind="ExternalOutput").ap()

    ag_in = nc.dram_tensor("ag_in", [OWN, F], bf16)
    ag_out = nc.dram_tensor("ag_out", [NPAD, F], bf16, addr_space="Shared")
    ztab = nc.dram_tensor("ztab", [NPAD, F], bf16)
    bn_in = nc.dram_tensor("bn_in", [F, 2], f32)
    bn_out = nc.dram_tensor("bn_out", [F, 2], f32, addr_space="Shared")

    nbmax = int(spc.max()) // 128

    with tile.TileContext(nc) as tc:
        with tc.tile_pool(name="const", bufs=1) as constp, \
             tc.tile_pool(name="big", bufs=1) as bigp, \
             tc.tile_pool(name="xso", bufs=4) as xsop, \
             tc.tile_pool(name="psA", bufs=2, space="PSUM") as psA, \
             tc.tile_pool(name="psB", bufs=2, space="PSUM") as psB, \
             tc.tile_pool(name="pss", bufs=1, space="PSUM") as pss, \
             tc.tile_pool(name="gbuf", bufs=3) as gbufp, \
             tc.tile_pool(name="oh", bufs=8) as ohp, \
             tc.tile_pool(name="wk", bufs=4) as wp, \
             tc.tile_pool(name="st", bufs=1) as stp:

            # ---- constants ----
            W1_t = constp.tile([F, F], bf16)
            W2_t = constp.tile([F, F], bf16)
            ident = constp.tile([128, 128], bf16)
            iota_b = constp.tile([128, 128], bf16)
            ones_c = constp.tile([128, 1], bf16)
            ones_r = constp.tile([1, F], f32)
            ones_rb = constp.tile([1, F], bf16)
            gamma_t = constp.tile([F, 1], f32)
            beta_t = constp.tile([F, 1], f32)
            b2r_t = constp.tile([1, F], f32)
            disT_t = constp.tile([128, CHUNKS], f32)
            nc.sync.dma_start(out=W1_t[:], in_=W1_d[:])
            nc.sync.dma_start(out=W2_t[:], in_=W2_d[:])
            nc.sync.dma_start(out=gamma_t[:], in_=gamma_d[:])
            nc.sync.dma_start(out=beta_t[:], in_=beta_d[:])
            nc.sync.dma_start(out=b2r_t[:], in_=b2r_d[:])
            nc.sync.dma_start(out=disT_t[:], in_=disT_d[:])
            make_identity(nc, ident[:])
            iota_i = constp.tile([128, 128], mybir.dt.int32)
            nc.gpsimd.iota(iota_i[:], pattern=[[1, 128]], base=0,
                           channel_multiplier=0)
            nc.vector.tensor_copy(out=iota_b[:], in_=iota_i[:])
            nc.vector.memset(ones_c[:], 1.0)
            nc.vector.memset(ones_r[:], 1.0)
            nc.vector.memset(ones_rb[:], 1.0)

            srcidx_sb = bigp.tile([128, tot_slots // 16], mybir.dt.int16)
            dstloc_sb = bigp.tile([128, ntiles], bf16)
            ncnt_sb = bigp.tile([1, consts["ncalls"]], mybir.dt.int32)
            nc.sync.dma_start(out=srcidx_sb[:], in_=srcidx_d[:])
            nc.sync.dma_start(out=dstloc_sb[:], in_=dstloc_d[:])
            nc.sync.dma_start(out=ncnt_sb[:], in_=ncnt_d[:])

            h1sb = bigp.tile([128, CHUNKS, 128], bf16)   # layer1 result, later z
            zTsb = bigp.tile([128, CHUNKS, 128], bf16)   # z transposed per chunk

            # b2 broadcast matrix (K=1 outer product ones x b2_row)
            b2m_ps = pss.tile([128, F], f32, tag="tmp")
            nc.tensor.matmul(out=b2m_ps[:], lhsT=ones_r[:], rhs=b2r_t[:],
                             start=True, stop=True)
            b2m_t = constp.tile([128, F], f32)
            nc.vector.tensor_copy(out=b2m_t[:], in_=b2m_ps[:])

            # ---- shared gather/scatter pass ----
            def layer_pass(table, out_stage, qn0):
                qn = qn0
                call_k = 0
                for c in range(CHUNKS):
                    nb = int(spc[c]) // 128
                    gb = gbufp.tile([128, nbmax, 128], bf16, tag="gb")
                    base_o = int(cell_off[c, 0])
                    for b in range(NBLK):
                        cnt = int(C[c, b])
                        o = int(cell_off[c, b])
                        lo = b * BLK
                        hi = min(NPAD, lo + BLK)
                        for sub in range(0, cnt, MAX_IDX_PER_CALL):
                            n = min(MAX_IDX_PER_CALL, cnt - sub)
                            ol = o - base_o + sub
                            cnt_reg = nc.values_load(
                                ncnt_sb[0:1, call_k:call_k + 1],
                                engines=[mybir.EngineType.Pool],
                                min_val=0, max_val=n,
                                skip_runtime_bounds_check=True)
                            call_k += 1
                            nc.gpsimd.dma_gather(
                                gb[:, ol // 128:(ol + n) // 128, :],
                                table[lo:hi, :],
                                srcidx_sb[:, (o + sub) // 16:(o + sub + n) // 16],
                                n, cnt_reg, F, queue_num=qn)
                            qn = (qn + 1) % 4
                    ps = psA.tile([128, 128], f32, tag="agg")
                    base_t = base_o // 128
                    for t in range(nb):
                        oh = ohp.tile([128, 128], bf16, tag="oh")
                        nc.vector.tensor_tensor(
                            out=oh[:],
                            in0=dstloc_sb[:, base_t + t:base_t + t + 1]
                                .to_broadcast([128, 128]),
                            in1=iota_b[:],
                            op=OP.is_equal)
                        # aggT[fin, dst] += gb_t.T @ oh
                        nc.tensor.matmul(out=ps[:], lhsT=gb[:, t, :],
                                         rhs=oh[:],
                                         start=(t == 0), stop=(t == nb - 1))
                    out_stage(c, ps)
                return qn

            # ---- layer 1 ----
            st_ps = pss.tile([128, 2], f32, tag="st2")
            sum_ps = st_ps[:, 0:1]
            sq_ps = st_ps[:, 1:2]

            def stage1(c, aggT_ps):
                aggT_sb = wp.tile([128, 128], bf16, tag="aggsb")
                nc.vector.tensor_copy(out=aggT_sb[:], in_=aggT_ps[:])
                h1_ps = psB.tile([128, 128], f32, tag="h")
                nc.tensor.matmul(out=h1_ps[:], lhsT=aggT_sb[:], rhs=W1_t[:],
                                 start=True, stop=False)
                xso_t = xsop.tile([F, 128], bf16, tag="xso")
                nc.sync.dma_start(out=xso_t[:],
                                  in_=xsoT_d[:, c * 128:(c + 1) * 128])
                nc.tensor.matmul(out=h1_ps[:], lhsT=xso_t[:], rhs=W1_t[:],
                                 start=False, stop=True)
                nc.vector.tensor_scalar_mul(out=h1sb[:, c, :], in0=h1_ps[:],
                                            scalar1=disT_t[:, c:c + 1])
                nc.tensor.matmul(out=sum_ps, lhsT=h1sb[:, c, :], rhs=ones_c[:],
                                 start=(c == 0), stop=(c == CHUNKS - 1))
                sq_sb = wp.tile([128, 128], bf16, tag="sq")
                nc.vector.tensor_tensor(out=sq_sb[:], in0=h1sb[:, c, :],
                                        in1=h1sb[:, c, :], op=OP.mult)
                nc.tensor.matmul(out=sq_ps, lhsT=sq_sb[:], rhs=ones_c[:],
                                 start=(c == 0), stop=(c == CHUNKS - 1))

            qn = layer_pass(xsn_d, stage1, 0)

            # ---- BN stats AllReduce + affine mats ----
            stats = stp.tile([128, 2], f32)
            nc.vector.tensor_copy(out=stats[:, 0:1], in_=sum_ps)
            nc.vector.tensor_copy(out=stats[:, 1:2], in_=sq_ps)
            nc.sync.dma_start(out=bn_in[:], in_=stats[:])
            nc.gpsimd.collective_compute(
                "AllReduce", OP.add, ins=[bn_in.ap()], outs=[bn_out.ap()],
                replica_groups=[list(range(NCORES))])
            gstats = stp.tile([128, 2], f32)
            nc.sync.dma_start(out=gstats[:], in_=bn_out[:])

            mean_t = stp.tile([128, 1], f32)
            ex2_t = stp.tile([128, 1], f32)
            var_t = stp.tile([128, 1], f32)
            sd_t = stp.tile([128, 1], f32)
            rstd_t = stp.tile([128, 1], f32)
            scale_c = stp.tile([128, 1], f32)
            shift_c = stp.tile([128, 1], f32)
            eps_t = stp.tile([128, 1], f32)
            nc.vector.tensor_scalar_mul(out=mean_t[:], in0=gstats[:, 0:1],
                                        scalar1=1.0 / N)
            nc.vector.tensor_scalar_mul(out=ex2_t[:], in0=gstats[:, 1:2],
                                        scalar1=1.0 / N)
            nc.vector.tensor_tensor(out=var_t[:], in0=mean_t[:], in1=mean_t[:],
                                    op=OP.mult)
            nc.vector.tensor_tensor(out=var_t[:], in0=ex2_t[:], in1=var_t[:],
                                    op=OP.subtract)
            nc.vector.memset(eps_t[:], BN_EPS)
            nc.scalar.activation(sd_t[:], var_t[:], AF.Sqrt, bias=eps_t[:])
            nc.vector.reciprocal(out=rstd_t[:], in_=sd_t[:])
            nc.vector.tensor_tensor(out=scale_c[:], in0=rstd_t[:], in1=gamma_t[:],
                                    op=OP.mult)
            nc.vector.tensor_tensor(out=shift_c[:], in0=mean_t[:], in1=scale_c[:],
                                    op=OP.mult)
            nc.vector.tensor_tensor(out=shift_c[:], in0=beta_t[:], in1=shift_c[:],
                                    op=OP.subtract)

            # scale/shift rows -> broadcast matrices via transpose + outer prod
            scale_cb = stp.tile([128, 1], bf16)
            shift_cb = stp.tile([128, 1], bf16)
            nc.vector.tensor_copy(out=scale_cb[:], in_=scale_c[:])
            nc.vector.tensor_copy(out=shift_cb[:], in_=shift_c[:])
            scT_ps = pss.tile([1, 128], bf16, tag="rowT")
            nc.tensor.transpose(out=scT_ps[:], in_=scale_cb[:], identity=ident[:])
            sc_row = stp.tile([1, 128], bf16)
            nc.vector.tensor_copy(out=sc_row[:], in_=scT_ps[:])
            shT_ps = pss.tile([1, 128], bf16, tag="rowT")
            nc.tensor.transpose(out=shT_ps[:], in_=shift_cb[:], identity=ident[:])
            sh_row = stp.tile([1, 128], bf16)
            nc.vector.tensor_copy(out=sh_row[:], in_=shT_ps[:])
            sm_ps = pss.tile([128, F], f32, tag="tmp")
            nc.tensor.matmul(out=sm_ps[:], lhsT=ones_rb[:], rhs=sc_row[:],
                             start=True, stop=True)
            scale_mat = constp.tile([128, F], bf16)
            nc.vector.tensor_copy(out=scale_mat[:], in_=sm_ps[:])
            sh_ps = pss.tile([128, F], f32, tag="tmp")
            nc.tensor.matmul(out=sh_ps[:], lhsT=ones_rb[:], rhs=sh_row[:],
                             start=True, stop=True)
            shift_mat = constp.tile([128, F], bf16)
            nc.vector.tensor_copy(out=shift_mat[:], in_=sh_ps[:])

            # ---- z pass: z = relu(dis*(scale_mat*h1 + shift_mat)) ----
            for c in range(CHUNKS):
                t1 = wp.tile([128, 128], bf16, tag="z1")
                nc.vector.tensor_tensor(out=t1[:], in0=h1sb[:, c, :],
                                        in1=scale_mat[:], op=OP.mult)
                t2 = wp.tile([128, 128], bf16, tag="z2")
                nc.vector.tensor_tensor(out=t2[:], in0=t1[:],
                                        in1=shift_mat[:], op=OP.add)
                nc.scalar.activation(h1sb[:, c, :], t2[:], AF.Relu,
                                     scale=disT_t[:, c:c + 1])
                nc.sync.dma_start(out=ag_in[c * 128:(c + 1) * 128, :],
                                  in_=h1sb[:, c, :])
                zT_ps = pss.tile([128, 128], bf16, tag="zT")
                nc.tensor.transpose(out=zT_ps[:], in_=h1sb[:, c, :],
                                    identity=ident[:])
                nc.vector.tensor_copy(out=zTsb[:, c, :], in_=zT_ps[:])

            nc.gpsimd.collective_compute(
                "AllGather", OP.bypass, ins=[ag_in.ap()], outs=[ag_out.ap()],
                replica_groups=[list(range(NCORES))])

            # bulk copy Shared -> local DRAM (gathering from Shared is slow)
            NCOPY = 16
            rows = NPAD // NCOPY
            for k in range(NCOPY):
                nc.sync.dma_start(
                    out=ztab[k * rows:(k + 1) * rows, :],
                    in_=ag_out[k * rows:(k + 1) * rows, :])

            # ---- layer 2 ----
            def stage2(c, aggT_ps):
                aggT_sb = wp.tile([128, 128], bf16, tag="aggsb")
                nc.vector.tensor_copy(out=aggT_sb[:], in_=aggT_ps[:])
                h2_ps = psB.tile([128, 128], f32, tag="h")
                nc.tensor.matmul(out=h2_ps[:], lhsT=aggT_sb[:], rhs=W2_t[:],
                                 start=True, stop=False)
                nc.tensor.matmul(out=h2_ps[:], lhsT=zTsb[:, c, :], rhs=W2_t[:],
                                 start=False, stop=True)
                t = wp.tile([128, 128], f32, tag="eo")
                nc.vector.tensor_scalar_mul(out=t[:], in0=h2_ps[:],
                                            scalar1=disT_t[:, c:c + 1])
                nc.vector.tensor_tensor(out=t[:], in0=t[:], in1=b2m_t[:],
                                        op=OP.add)
                nc.scalar.activation(t[:], t[:], AF.Relu)
                nc.sync.dma_start(out=out_d[c * 128:(c + 1) * 128, :], in_=t[:])

            layer_pass(ztab.ap(), stage2, qn)

    nc.compile()
    return nc


def kernel(**inputs):
    global LAST_EXEC_NS
    import os
    x = inputs["x"]
    W1 = np.asarray(inputs["W1"], dtype=np.float32)
    W2 = np.asarray(inputs["W2"], dtype=np.float32)
    gamma = np.asarray(inputs["gamma"], dtype=np.float32)
    beta = np.asarray(inputs["beta"], dtype=np.float32)
    b2 = np.asarray(inputs["b2"], dtype=np.float32)
    edge_index = inputs["edge_index"]

    key = (hash(np.asarray(edge_index)[:, ::997].tobytes()),)
    if key not in _cache:
        consts, xsn_bf, per_core = _prep(x, edge_index)
        nc = _build(consts)
        _cache[key] = (consts, nc)
    else:
        consts, nc = _cache[key]
        _, xsn_bf, per_core = _prep(x, edge_index)

    shared = {
        "xsn": xsn_bf,
        "W1bf": W1.astype(BF16), "W2bf": W2.astype(BF16),
        "gamma_c": gamma.reshape(F, 1).copy(),
        "beta_c": beta.reshape(F, 1).copy(),
        "b2_row": b2.reshape(1, F).copy(),
    }
    in_maps = []
    for i in range(NCORES):
        m = dict(shared)
        m.update(per_core[i])
        in_maps.append(m)

    trace = bool(os.environ.get("BASS_GCN_TRACE"))
    res = run_bass_kernel_spmd(nc, in_maps, list(range(NCORES)), trace=trace)
    LAST_EXEC_NS = res.exec_time_ns

    out = np.concatenate([res.results[i]["out"] for i in range(NCORES)], axis=0)
    return np.ascontiguousarray(out[:N]).astype(np.float32)


# revision 14
# speedup vs baseline: 1.9044x; 1.0180x over previous
"""2-layer GCN (GCNConv -> BatchNorm(train) -> ReLU -> GCNConv -> ReLU) on 8 TRN2
NeuronCores, SPMD (one NEFF on all cores).

v2 design (aggregate-then-matmul, bf16 datapath):
  - Host provides xsn = (dis*x) as a bf16 [NPAD,128] gather table; layer-1
    gathers xsn rows per edge directly (no H1-table build phase at all) and
    the W1 matmul is applied per dst-chunk AFTER aggregation.
  - Edges partitioned by dst owner, sorted by (chunk, src-block, src); the
    segment-sum scatter is a one-hot matmul per 128-row tile producing the
    aggregate TRANSPOSED ([fin, dst]) so the @W chain needs no transposes.
  - Padding slots carry idx=-1: the DMAGatherAnt ucode skips trailing
    negative indices per call, so padding costs no Q7 descriptor-gen time
    and no DMA bytes (except the first 3 chunks, which zero-pad to
    initialize the rotating gather buffers).
  - Self-loop terms are folded in as an extra accumulating matmul with a
    feature-major own-shard slice as lhsT.
  - BN stats via ones-matmuls accumulated across chunks + 1KB AllReduce;
    scale/shift folded as z = relu(dis * (scale_mat*h1 + shift_mat)) with
    scale/shift broadcast matrices built by K=1 outer-product matmuls.
  - z table (bf16) AllGathered, bulk-copied Shared->local DRAM, then
    layer 2 runs the same gather/scatter machinery; W2 + self term + b2 +
    relu applied per chunk; f32 output rows.
"""
import numpy as np
import ml_dtypes

import concourse.bass as bass
import concourse.mybir as mybir
import concourse.tile as tile
from concourse import bacc
from concourse.bass_utils import run_bass_kernel_spmd
from concourse.masks import make_identity

N = 100000
F = 128
NCORES = 8
NPAD = 102400
OWN = NPAD // NCORES          # 12800
CHUNKS = OWN // 128           # 100
BLK = 32768
NBLK = 4
BN_EPS = 1e-5
MAX_IDX_PER_CALL = 2048
ZPAD_CHUNKS = 3               # first chunks zero-pad (init gather bufs)

BF16 = ml_dtypes.bfloat16

LAST_EXEC_NS = None
_cache = {}


def _prep(x, edge_index):
    src = np.asarray(edge_index[0]).astype(np.int64)
    dst = np.asarray(edge_index[1]).astype(np.int64)

    deg = np.bincount(dst, minlength=N).astype(np.float32) + 1.0
    dis = np.zeros(NPAD, dtype=np.float32)
    dis[:N] = 1.0 / np.sqrt(deg)

    xsn = np.zeros((NPAD, F), dtype=np.float32)
    xsn[:N] = np.asarray(x, dtype=np.float32) * dis[:N, None]
    xsn_bf = xsn.astype(BF16)

    owner = dst // OWN
    chunk = (dst % OWN) // 128
    blk = src // BLK
    cell = ((owner * CHUNKS + chunk) * NBLK + blk).astype(np.int64)
    order = np.lexsort((src, cell))
    cell_s = cell[order]
    src_s = src[order]
    dstloc_s = (dst[order] % 128).astype(np.float32)

    counts = np.bincount(cell_s, minlength=NCORES * CHUNKS * NBLK)
    counts = counts.reshape(NCORES, CHUNKS, NBLK)
    C = counts.max(axis=0)
    C = ((C + 127) // 128) * 128
    C = np.maximum(C, 128)
    slots_per_chunk = C.sum(axis=1)
    tot_slots = int(slots_per_chunk.sum())
    ntiles = tot_slots // 128

    cell_off = np.zeros((CHUNKS, NBLK), dtype=np.int64)
    cell_off.reshape(-1)[1:] = np.cumsum(C.reshape(-1))[:-1]

    starts = np.zeros(NCORES * CHUNKS * NBLK + 1, dtype=np.int64)
    starts[1:] = np.cumsum(counts.reshape(-1))

    per_core = []
    for i in range(NCORES):
        srcidx = np.full(tot_slots, -1, dtype=np.int16)
        dstloc = np.full(tot_slots, -1.0, dtype=np.float32)
        for c in range(CHUNKS):
            for b in range(NBLK):
                k = (i * CHUNKS + c) * NBLK + b
                n = int(counts[i, c, b])
                o = int(cell_off[c, b])
                if c < ZPAD_CHUNKS:
                    srcidx[o:o + C[c, b]] = 0   # zero-pad: real row-0 gathers
                if n:
                    sl = slice(starts[k], starts[k] + n)
                    srcidx[o:o + n] = (src_s[sl] - b * BLK).astype(np.int16)
                    dstloc[o:o + n] = dstloc_s[sl]
        iw = srcidx.reshape(tot_slots // 16, 16).T            # [16, tot/16]
        srcidx_w = np.ascontiguousarray(np.tile(iw, (8, 1)))  # [128, tot/16]
        dstloc_t = np.ascontiguousarray(
            dstloc.reshape(ntiles, 128).T).astype(BF16)
        disT = np.ascontiguousarray(
            dis[i * OWN:(i + 1) * OWN].reshape(CHUNKS, 128).T)
        xsoT = np.ascontiguousarray(
            xsn_bf[i * OWN:(i + 1) * OWN].T)                  # [F, OWN] bf16
        per_core.append({"srcidx": srcidx_w, "dstloc": dstloc_t,
                         "disT": disT, "xsoT": xsoT})

    ncalls = 0
    for c in range(CHUNKS):
        for b in range(NBLK):
            for sub in range(0, int(C[c, b]), MAX_IDX_PER_CALL):
                ncalls += 1
    for i in range(NCORES):
        ncnt = np.zeros((1, ncalls), dtype=np.int32)
        k = 0
        for c in range(CHUNKS):
            for b in range(NBLK):
                cnt = int(C[c, b])
                nreal = cnt if c < ZPAD_CHUNKS else int(counts[i, c, b])
                for sub in range(0, cnt, MAX_IDX_PER_CALL):
                    n = min(MAX_IDX_PER_CALL, cnt - sub)
                    ncnt[0, k] = min(max(nreal - sub, 0), n)
                    k += 1
        per_core[i]["ncnt"] = ncnt

    consts = {"C": C, "cell_off": cell_off, "tot_slots": tot_slots,
              "ntiles": ntiles, "slots_per_chunk": slots_per_chunk,
              "ncalls": ncalls}
    return consts, xsn_bf, per_core


def _build(consts):
    C = consts["C"]
    cell_off = consts["cell_off"]
    tot_slots = consts["tot_slots"]
    ntiles = consts["ntiles"]
    spc = consts["slots_per_chunk"]

    f32 = mybir.dt.float32
    bf16 = mybir.dt.bfloat16
    AF = mybir.ActivationFunctionType
    OP = mybir.AluOpType
    nc = bacc.Bacc("TRN2", target_bir_lowering=False, debug=False,
                   num_devices=NCORES, num_swdge_queues=4)

    xsn_d = nc.dram_tensor("xsn", [NPAD, F], bf16, kind="ExternalInput").ap()
    xsoT_d = nc.dram_tensor("xsoT", [F, OWN], bf16, kind="ExternalInput").ap()
    W1_d = nc.dram_tensor("W1bf", [F, F], bf16, kind="ExternalInput").ap()
    W2_d = nc.dram_tensor("W2bf", [F, F], bf16, kind="ExternalInput").ap()
    gamma_d = nc.dram_tensor("gamma_c", [F, 1], f32, kind="ExternalInput").ap()
    beta_d = nc.dram_tensor("beta_c", [F, 1], f32, kind="ExternalInput").ap()
    b2r_d = nc.dram_tensor("b2_row", [1, F], f32, kind="ExternalInput").ap()
    disT_d = nc.dram_tensor("disT", [128, CHUNKS], f32, kind="ExternalInput").ap()
    srcidx_d = nc.dram_tensor("srcidx", [128, tot_slots // 16], mybir.dt.int16,
                              kind="ExternalInput").ap()
    dstloc_d = nc.dram_tensor("dstloc", [128, ntiles], bf16,
                              kind="ExternalInput").ap()
    ncnt_d = nc.dram_tensor("ncnt", [1, consts["ncalls"]], mybir.dt.int32,
                            kind="ExternalInput").ap()
    out_d = nc.dram_tensor("out", [OWN, F], f32, kind="ExternalOutput").ap()

    ag_in = nc.dram_tensor("ag_in", [OWN, F], bf16)
    ag_out = nc.dram_tensor("ag_out", [NPAD, F], bf16, addr_space="Shared")
    bn_in = nc.dram_tensor("bn_in", [F, 2], f32)
    bn_out = nc.dram_tensor("bn_out", [F, 2], f32, addr_space="Shared")

    nbmax = int(spc.max()) // 128

    with tile.TileContext(nc) as tc:
        with tc.tile_pool(name="const", bufs=1) as constp, \
             tc.tile_pool(name="big", bufs=1) as bigp, \
             tc.tile_pool(name="xso", bufs=4) as xsop, \
             tc.tile_pool(name="psA", bufs=2, space="PSUM") as psA, \
             tc.tile_pool(name="psB", bufs=2, space="PSUM") as psB, \
             tc.tile_pool(name="pss", bufs=1, space="PSUM") as pss, \
             tc.tile_pool(name="gbuf", bufs=3) as gbufp, \
             tc.tile_pool(name="oh", bufs=8) as ohp, \
             tc.tile_pool(name="wk", bufs=4) as wp, \
             tc.tile_pool(name="st", bufs=1) as stp:

            # ---- constants ----
            W1_t = constp.tile([F, F], bf16)
            W2_t = constp.tile([F, F], bf16)
            ident = constp.tile([128, 128], bf16)
            iota_b = constp.tile([128, 128], bf16)
            ones_c = constp.tile([128, 1], bf16)
            ones_r = constp.tile([1, F], f32)
            ones_rb = constp.tile([1, F], bf16)
            gamma_t = constp.tile([F, 1], f32)
            beta_t = constp.tile([F, 1], f32)
            b2r_t = constp.tile([1, F], f32)
            disT_t = constp.tile([128, CHUNKS], f32)
            nc.sync.dma_start(out=W1_t[:], in_=W1_d[:])
            nc.sync.dma_start(out=W2_t[:], in_=W2_d[:])
            nc.sync.dma_start(out=gamma_t[:], in_=gamma_d[:])
            nc.sync.dma_start(out=beta_t[:], in_=beta_d[:])
            nc.sync.dma_start(out=b2r_t[:], in_=b2r_d[:])
            nc.sync.dma_start(out=disT_t[:], in_=disT_d[:])
            make_identity(nc, ident[:])
            iota_i = constp.tile([128, 128], mybir.dt.int32)
            nc.gpsimd.iota(iota_i[:], pattern=[[1, 128]], base=0,
                           channel_multiplier=0)
            nc.vector.tensor_copy(out=iota_b[:], in_=iota_i[:])
            nc.vector.memset(ones_c[:], 1.0)
            nc.vector.memset(ones_r[:], 1.0)
            nc.vector.memset(ones_rb[:], 1.0)

            srcidx_sb = bigp.tile([128, tot_slots // 16], mybir.dt.int16)
            dstloc_sb = bigp.tile([128, ntiles], bf16)
            ncnt_sb = bigp.tile([1, consts["ncalls"]], mybir.dt.int32)
            nc.sync.dma_start(out=srcidx_sb[:], in_=srcidx_d[:])
            nc.sync.dma_start(out=dstloc_sb[:], in_=dstloc_d[:])
            nc.sync.dma_start(out=ncnt_sb[:], in_=ncnt_d[:])

            h1sb = bigp.tile([128, CHUNKS, 128], bf16)   # layer1 result, later z
            zTsb = bigp.tile([128, CHUNKS, 128], bf16)   # z transposed per chunk

            # b2 broadcast matrix (K=1 outer product ones x b2_row)
            b2m_ps = pss.tile([128, F], f32, tag="tmp")
            nc.tensor.matmul(out=b2m_ps[:], lhsT=ones_r[:], rhs=b2r_t[:],
                             start=True, stop=True)
            b2m_t = constp.tile([128, F], f32)
            nc.vector.tensor_copy(out=b2m_t[:], in_=b2m_ps[:])

            # ---- shared gather/scatter pass ----
            def layer_pass(table, out_stage, qn0):
                qn = qn0
                call_k = 0
                for c in range(CHUNKS):
                    nb = int(spc[c]) // 128
                    gb = gbufp.tile([128, nbmax, 128], bf16, tag="gb")
                    base_o = int(cell_off[c, 0])
                    for b in range(NBLK):
                        cnt = int(C[c, b])
                        o = int(cell_off[c, b])
                        lo = b * BLK
                        hi = min(NPAD, lo + BLK)
                        for sub in range(0, cnt, MAX_IDX_PER_CALL):
                            n = min(MAX_IDX_PER_CALL, cnt - sub)
                            ol = o - base_o + sub
                            cnt_reg = nc.values_load(
                                ncnt_sb[0:1, call_k:call_k + 1],
                                engines=[mybir.EngineType.Pool],
                                min_val=0, max_val=n,
                                skip_runtime_bounds_check=True)
                            call_k += 1
                            nc.gpsimd.dma_gather(
                                gb[:, ol // 128:(ol + n) // 128, :],
                                table[lo:hi, :],
                                srcidx_sb[:, (o + sub) // 16:(o + sub + n) // 16],
                                n, cnt_reg, F, queue_num=qn)
                            qn = (qn + 1) % 4
                    ps = psA.tile([128, 128], f32, tag="agg")
                    base_t = base_o // 128
                    for t in range(nb):
                        oh = ohp.tile([128, 128], bf16, tag="oh")
                        nc.vector.tensor_tensor(
                            out=oh[:],
                            in0=dstloc_sb[:, base_t + t:base_t + t + 1]
                                .to_broadcast([128, 128]),
                            in1=iota_b[:],
                            op=OP.is_equal)
                        # aggT[fin, dst] += gb_t.T @ oh
                        nc.tensor.matmul(out=ps[:], lhsT=gb[:, t, :],
                                         rhs=oh[:],
                                         start=(t == 0), stop=(t == nb - 1))
                    out_stage(c, ps)
                return qn

            # ---- layer 1 ----
            st_ps = pss.tile([128, 2], f32, tag="st2")
            sum_ps = st_ps[:, 0:1]
            sq_ps = st_ps[:, 1:2]

            def stage1(c, aggT_ps):
                aggT_sb = wp.tile([128, 128], bf16, tag="aggsb")
                nc.vector.tensor_copy(out=aggT_sb[:], in_=aggT_ps[:])
                h1_ps = psB.tile([128, 128], f32, tag="h")
                nc.tensor.matmul(out=h1_ps[:], lhsT=aggT_sb[:], rhs=W1_t[:],
                                 start=True, stop=False)
                xso_t = xsop.tile([F, 128], bf16, tag="xso")
                nc.sync.dma_start(out=xso_t[:],
                                  in_=xsoT_d[:, c * 128:(c + 1) * 128])
                nc.tensor.matmul(out=h1_ps[:], lhsT=xso_t[:], rhs=W1_t[:],
                                 start=False, stop=True)
                nc.vector.tensor_scalar_mul(out=h1sb[:, c, :], in0=h1_ps[:],
                                            scalar1=disT_t[:, c:c + 1])
                nc.tensor.matmul(out=sum_ps, lhsT=h1sb[:, c, :], rhs=ones_c[:],
                                 start=(c == 0), stop=(c == CHUNKS - 1))
                sq_sb = wp.tile([128, 128], bf16, tag="sq")
                nc.vector.tensor_tensor(out=sq_sb[:], in0=h1sb[:, c, :],
                                        in1=h1sb[:, c, :], op=OP.mult)
                nc.tensor.matmul(out=sq_ps, lhsT=sq_sb[:], rhs=ones_c[:],
                                 start=(c == 0), stop=(c == CHUNKS - 1))

            qn = layer_pass(xsn_d, stage1, 0)

            # ---- BN stats AllReduce + affine mats ----
            stats = stp.tile([128, 2], f32)
            nc.vector.tensor_copy(out=stats[:, 0:1], in_=sum_ps)
            nc.vector.tensor_copy(out=stats[:, 1:2], in_=sq_ps)
            nc.sync.dma_start(out=bn_in[:], in_=stats[:])
            nc.gpsimd.collective_compute(
                "AllReduce", OP.add, ins=[bn_in.ap()], outs=[bn_out.ap()],
                replica_groups=[list(range(NCORES))])
            gstats = stp.tile([128, 2], f32)
            nc.sync.dma_start(out=gstats[:], in_=bn_out[:])

            mean_t = stp.tile([128, 1], f32)
            ex2_t = stp.tile([128, 1], f32)
            var_t = stp.tile([128, 1], f32)
            sd_t = stp.tile([128, 1], f32)
            rstd_t = stp.tile([128, 1], f32)
            scale_c = stp.tile([128, 1], f32)
            shift_c = stp.tile([128, 1], f32)
            eps_t = stp.tile([128, 1], f32)
            nc.vector.tensor_scalar_mul(out=mean_t[:], in0=gstats[:, 0:1],
                                        scalar1=1.0 / N)
            nc.vector.tensor_scalar_mul(out=ex2_t[:], in0=gstats[:, 1:2],
                                        scalar1=1.0 / N)
            nc.vector.tensor_tensor(out=var_t[:], in0=mean_t[:], in1=mean_t[:],
                                    op=OP.mult)
            nc.vector.tensor_tensor(out=var_t[:], in0=ex2_t[:], in1=var_t[:],
                                    op=OP.subtract)
            nc.vector.memset(eps_t[:], BN_EPS)
            nc.scalar.activation(sd_t[:], var_t[:], AF.Sqrt, bias=eps_t[:])
            nc.vector.reciprocal(out=rstd_t[:], in_=sd_t[:])
            nc.vector.tensor_tensor(out=scale_c[:], in0=rstd_t[:], in1=gamma_t[:],
                                    op=OP.mult)
            nc.vector.tensor_tensor(out=shift_c[:], in0=mean_t[:], in1=scale_c[:],
                                    op=OP.mult)
            nc.vector.tensor_tensor(out=shift_c[:], in0=beta_t[:], in1=shift_c[:],
                                    op=OP.subtract)

            # scale/shift rows -> broadcast matrices via transpose + outer prod
            scale_cb = stp.tile([128, 1], bf16)
            shift_cb = stp.tile([128, 1], bf16)
            nc.vector.tensor_copy(out=scale_cb[:], in_=scale_c[:])
            nc.vector.tensor_copy(out=shift_cb[:], in_=shift_c[:])
            scT_ps = pss.tile([1, 128], bf16, tag="rowT")
            nc.tensor.transpose(out=scT_ps[:], in_=scale_cb[:], identity=ident[:])
            sc_row = stp.tile([1, 128], bf16)
            nc.vector.tensor_copy(out=sc_row[:], in_=scT_ps[:])
            shT_ps = pss.tile([1, 128], bf16, tag="rowT")
            nc.tensor.transpose(out=shT_ps[:], in_=shift_cb[:], identity=ident[:])
            sh_row = stp.tile([1, 128], bf16)
            nc.vector.tensor_copy(out=sh_row[:], in_=shT_ps[:])
            sm_ps = pss.tile([128, F], f32, tag="tmp")
            nc.tensor.matmul(out=sm_ps[:], lhsT=ones_rb[:], rhs=sc_row[:],
                             start=True, stop=True)
            scale_mat = constp.tile([128, F], bf16)
            nc.vector.tensor_copy(out=scale_mat[:], in_=sm_ps[:])
            sh_ps = pss.tile([128, F], f32, tag="tmp")
            nc.tensor.matmul(out=sh_ps[:], lhsT=ones_rb[:], rhs=sh_row[:],
                             start=True, stop=True)
            shift_mat = constp.tile([128, F], bf16)
            nc.vector.tensor_copy(out=shift_mat[:], in_=sh_ps[:])

            # ---- z pass: z = relu(dis*(scale_mat*h1 + shift_mat)) ----
            for c in range(CHUNKS):
                t1 = wp.tile([128, 128], bf16, tag="z1")
                nc.vector.tensor_tensor(out=t1[:], in0=h1sb[:, c, :],
                                        in1=scale_mat[:], op=OP.mult)
                t2 = wp.tile([128, 128], bf16, tag="z2")
                nc.vector.tensor_tensor(out=t2[:], in0=t1[:],
                                        in1=shift_mat[:], op=OP.add)
                nc.scalar.activation(h1sb[:, c, :], t2[:], AF.Relu,
                                     scale=disT_t[:, c:c + 1])
                nc.sync.dma_start(out=ag_in[c * 128:(c + 1) * 128, :],
                                  in_=h1sb[:, c, :])
                zT_ps = pss.tile([128, 128], bf16, tag="zT")
                nc.tensor.transpose(out=zT_ps[:], in_=h1sb[:, c, :],
                                    identity=ident[:])
                nc.vector.tensor_copy(out=zTsb[:, c, :], in_=zT_ps[:])

            nc.gpsimd.collective_compute(
                "AllGather", OP.bypass, ins=[ag_in.ap()], outs=[ag_out.ap()],
                replica_groups=[list(range(NCORES))])

            # ---- layer 2 ----
            def stage2(c, aggT_ps):
                aggT_sb = wp.tile([128, 128], bf16, tag="aggsb")
                nc.vector.tensor_copy(out=aggT_sb[:], in_=aggT_ps[:])
                h2_ps = psB.tile([128, 128], f32, tag="h")
                nc.tensor.matmul(out=h2_ps[:], lhsT=aggT_sb[:], rhs=W2_t[:],
                                 start=True, stop=False)
                nc.tensor.matmul(out=h2_ps[:], lhsT=zTsb[:, c, :], rhs=W2_t[:],
                                 start=False, stop=True)
                t = wp.tile([128, 128], f32, tag="eo")
                nc.vector.tensor_scalar_mul(out=t[:], in0=h2_ps[:],
                                            scalar1=disT_t[:, c:c + 1])
                nc.vector.tensor_tensor(out=t[:], in0=t[:], in1=b2m_t[:],
                                        op=OP.add)
                nc.scalar.activation(t[:], t[:], AF.Relu)
                nc.sync.dma_start(out=out_d[c * 128:(c + 1) * 128, :], in_=t[:])

            layer_pass(ag_out.ap(), stage2, qn)

    nc.compile()
    return nc


def kernel(**inputs):
    global LAST_EXEC_NS
    import os
    x = inputs["x"]
    W1 = np.asarray(inputs["W1"], dtype=np.float32)
    W2 = np.asarray(inputs["W2"], dtype=np.float32)
    gamma = np.asarray(inputs["gamma"], dtype=np.float32)
    beta = np.asarray(inputs["beta"], dtype=np.float32)
    b2 = np.asarray(inputs["b2"], dtype=np.float32)
    edge_index = inputs["edge_index"]

    key = (hash(np.asarray(edge_index)[:, ::997].tobytes()),)
    if key not in _cache:
        consts, xsn_bf, per_core = _prep(x, edge_index)
        nc = _build(consts)
        _cache[key] = (consts, nc)
    else:
        consts, nc = _cache[key]
        _, xsn_bf, per_core = _prep(x, edge_index)

    shared = {
        "xsn": xsn_bf,
        "W1bf": W1.astype(BF16), "W2bf": W2.astype(BF16),
        "gamma_c": gamma.reshape(F, 1).copy(),
        "beta_c": beta.reshape(F, 1).copy(),
        "b2_row": b2.reshape(1, F).copy(),
    }
    in_maps = []
    for i in range(NCORES):
        m = dict(shared)
        m.update(per_core[i])
        in_maps.append(m)

    trace = bool(os.environ.get("BASS_GCN_TRACE"))
    res = run_bass_kernel_spmd(nc, in_maps, list(range(NCORES)), trace=trace)
    LAST_EXEC_NS = res.exec_time_ns

    out = np.concatenate([res.results[i]["out"] for i in range(NCORES)], axis=0)
    return np.ascontiguousarray(out[:N]).astype(np.float32)
